# revision 30
# baseline (speedup 1.0000x reference)
import sys

if "/opt/trn_rl_repo" not in sys.path:
    sys.path.insert(0, "/opt/trn_rl_repo")

import heapq

import numpy as np

B, T, C = 2, 2048, 2048
H, H_KV = 16, 8
D = C // H  # 128
NCORES = 8
HL = H // NCORES  # 2 local query heads per core; 1 kv head per core

F32R_SCALE = 0.08838834764831845  # 1/sqrt(128)


def build_nc(b=B, t=T, c=C, mmdt="bf16", debug=False):
    """Per-core Bass program. Same program on all 8 cores; the sharding
    lives entirely in the input data each core receives.

    Schedule (all engines in-order, so emission order = execution order):
      phase A: proj(b0) quarter by quarter (qq pass then kv pass per i4)
      phase B: attention strips of b0 (descending i4) woven with proj(b1)
               quarters and b0 output-projection blocks
      phase C: attention strips of b1 (descending i4) woven with the
               remaining output-projection blocks
    The weave keeps the PE queue stocked with independent matmuls so
    dependency stalls (exp latency, psum evictions) never idle the PE.
    """
    import concourse.bass as bass  # noqa: F401
    import concourse.mybir as mybir
    import concourse.tile as tile
    from concourse import bacc

    f32 = mybir.dt.float32
    bf = mybir.dt.float32r if mmdt == "f32r" else mybir.dt.bfloat16
    EXP = mybir.ActivationFunctionType.Exp

    ncb = c // 128  # contraction blocks for projections
    nt = t // 512  # number of 512-wide t quarters
    swap_mask = [i ^ 1 for i in range(32)]

    nc = bacc.Bacc("TRN2", target_bir_lowering=False, debug=False)

    # weights arrive pre-laid-out for SBUF: [128 partitions, chunked cols]
    xT = nc.dram_tensor("xT", [b, c, t], bf, kind="ExternalInput")
    wq = nc.dram_tensor("wq", [128, (c // 128) * HL * D], bf, kind="ExternalInput")
    wk = nc.dram_tensor("wk", [128, (c // 128) * D], bf, kind="ExternalInput")
    wv = nc.dram_tensor("wv", [128, (c // 128) * D], bf, kind="ExternalInput")
    wp = nc.dram_tensor("wp", [128, HL * c], bf, kind="ExternalInput")
    cos2 = nc.dram_tensor("cos2", [128, t], f32, kind="ExternalInput")
    sin2 = nc.dram_tensor("sin2", [128, t], f32, kind="ExternalInput")
    maskf = nc.dram_tensor("maskf", [128, 128], bf, kind="ExternalInput")
    onesv = nc.dram_tensor("onesv", [128, 1], bf, kind="ExternalInput")
    ident = nc.dram_tensor("ident", [128, 128], f32, kind="ExternalInput")
    y = nc.dram_tensor("y", [b, t, c], bf, kind="ExternalOutput")
    if debug:
        dbg_qt = nc.dram_tensor("dbg_qt", [b, HL, 128, t], bf, kind="ExternalOutput")
        dbg_kt = nc.dram_tensor("dbg_kt", [b, 128, t], bf, kind="ExternalOutput")
        dbg_vn = nc.dram_tensor("dbg_vn", [b, 128, t], bf, kind="ExternalOutput")
        dbg_at = nc.dram_tensor("dbg_at", [b, HL, 128, t], bf, kind="ExternalOutput")

    with tile.TileContext(nc) as tc:
        with (
            tc.tile_pool(name="wts", bufs=1) as wpool,
            tc.tile_pool(name="data", bufs=1) as dpool,
            tc.tile_pool(name="work", bufs=2) as wkp,
            tc.tile_pool(name="psum", bufs=1, space="PSUM") as pp,
        ):
            # ---- weight/table DMAs, in first-use priority order ----
            nw = 4  # cb chunks per weight DMA group
            ngrp = ncb // nw
            wq_sbs = [wpool.tile([128, nw * HL * D], bf, name=f"wq{g}") for g in range(ngrp)]
            wk_sbs = [wpool.tile([128, nw * D], bf, name=f"wk{g}") for g in range(ngrp)]
            wv_sbs = [wpool.tile([128, nw * D], bf, name=f"wv{g}") for g in range(ngrp)]
            cosq = [wpool.tile([128, 512], f32, name=f"cosq{i}") for i in range(nt)]
            sinq = [wpool.tile([128, 512], f32, name=f"sinq{i}") for i in range(nt)]
            id_sb = wpool.tile([128, 128], f32)
            mask_sb = wpool.tile([128, 128], bf)
            ones_sb = wpool.tile([128, 1], bf)
            wp_sb = wpool.tile([128, HL * c], bf)

            def dma_wgrp(g):
                nc.scalar.dma_start(
                    wq_sbs[g][:], wq[:, g * nw * HL * D : (g + 1) * nw * HL * D]
                )
                nc.scalar.dma_start(wk_sbs[g][:], wk[:, g * nw * D : (g + 1) * nw * D])
                nc.scalar.dma_start(wv_sbs[g][:], wv[:, g * nw * D : (g + 1) * nw * D])

            # first 256 cols of wq unblock the PE earliest; wk/wv follow
            # since the kv pass trails the qq pass by a full quarter
            nc.scalar.dma_start(wq_sbs[0][:, 0:256], wq[:, 0:256])
            nc.scalar.dma_start(wq_sbs[0][:, 256:], wq[:, 256 : nw * HL * D])
            nc.scalar.dma_start(wq_sbs[1][:], wq[:, nw * HL * D : 2 * nw * HL * D])
            nc.scalar.dma_start(wk_sbs[0][:], wk[:, 0 : nw * D])
            nc.scalar.dma_start(wv_sbs[0][:], wv[:, 0 : nw * D])
            for g in (2, 3):
                nc.scalar.dma_start(
                    wq_sbs[g][:], wq[:, g * nw * HL * D : (g + 1) * nw * HL * D]
                )
            nc.scalar.dma_start(cosq[0][:], cos2[:, 0:512])
            nc.scalar.dma_start(sinq[0][:], sin2[:, 0:512])
            for g in range(1, ngrp):
                nc.scalar.dma_start(wk_sbs[g][:], wk[:, g * nw * D : (g + 1) * nw * D])
                nc.scalar.dma_start(wv_sbs[g][:], wv[:, g * nw * D : (g + 1) * nw * D])
            nc.scalar.dma_start(id_sb[:], ident[:, :])
            for i in range(1, nt):
                nc.scalar.dma_start(cosq[i][:], cos2[:, i * 512 : (i + 1) * 512])
                nc.scalar.dma_start(sinq[i][:], sin2[:, i * 512 : (i + 1) * 512])
            nc.scalar.dma_start(mask_sb[:], maskf[:, :])
            nc.scalar.dma_start(ones_sb[:], onesv[:, :])
            nc.scalar.dma_start(wp_sb[:], wp[:, :])
            warm = wpool.tile([128, 1], f32)
            nc.scalar.activation(warm[:], id_sb[:, 0:1], EXP, scale=1.0)

            # ---- per-batch persistent tiles ----
            QT = [[dpool.tile([128, t], bf, name=f"QT{bi}_{h}") for h in range(HL)] for bi in range(b)]
            KT = [dpool.tile([128, t], bf, name=f"KT{bi}") for bi in range(b)]
            Vn = [dpool.tile([128, t], bf, name=f"Vn{bi}") for bi in range(b)]
            AT = [[dpool.tile([128, t], bf, name=f"AT{bi}_{h}") for h in range(HL)] for bi in range(b)]

            def rope(ps, dest, i4):
                """dest[:, quarter] = ps*cos + swap_adjacent(ps)*sin"""
                ts_ = slice(i4 * 512, (i4 + 1) * 512)
                ra = wkp.tile([128, 512], f32, tag="ra", bufs=2, name="ra")
                rb = wkp.tile([128, 512], f32, tag="rb", bufs=2, name="rb")
                nc.vector.tensor_mul(ra[:], ps[:], cosq[i4][:])
                nc.vector.stream_shuffle(rb[:], ps[:], swap_mask)
                nc.vector.tensor_mul(rb[:], rb[:], sinq[i4][:])
                nc.vector.tensor_add(dest[:, ts_], ra[:], rb[:])

            def proj_quarter(bi, i4, dual_ring=False):
                """Project x[:, quarter i4] -> QT (roped), KT (roped), Vn."""
                ts_ = slice(i4 * 512, (i4 + 1) * 512)
                xts = []
                for cb in range(ncb):
                    xtp = wkp.tile([128, 512], bf, tag="xt", bufs=32, name=f"xt{bi}_{i4}_{cb}")
                    eng = nc.gpsimd if dual_ring and cb % 2 else nc.sync
                    eng.dma_start(xtp[:], xT[bi, cb * 128 : (cb + 1) * 128, ts_])
                    xts.append(xtp)
                # pass 1: the two q heads
                pq0 = pp.tile([128, 512], f32, tag="proj", bufs=2, name="pq0")
                pq1 = pp.tile([128, 512], f32, tag="proj", bufs=2, name="pq1")
                for cb in range(ncb):
                    g, cbl = cb // nw, cb % nw
                    base = cbl * HL * D
                    st, sp = (cb == 0), (cb == ncb - 1)
                    nc.tensor.matmul(
                        pq0[:], wq_sbs[g][:, base : base + 128], xts[cb][:],
                        start=st, stop=sp, skip_group_check=True,
                    )
                    nc.tensor.matmul(
                        pq1[:], wq_sbs[g][:, base + 128 : base + 256], xts[cb][:],
                        start=st, stop=sp, skip_group_check=True,
                    )
                    yield
                rope(pq0, QT[bi][0], i4)
                rope(pq1, QT[bi][1], i4)
                yield
                # pass 2: k (roped) and v (transposed into Vn)
                pk = pp.tile([128, 512], f32, tag="proj", bufs=2, name="pk")
                pv = pp.tile([128, 512], f32, tag="proj", bufs=2, name="pv")
                for cb in range(ncb):
                    g, cbl = cb // nw, cb % nw
                    st, sp = (cb == 0), (cb == ncb - 1)
                    nc.tensor.matmul(
                        pk[:], wk_sbs[g][:, cbl * 128 : (cbl + 1) * 128], xts[cb][:],
                        start=st, stop=sp, skip_group_check=True,
                    )
                    nc.tensor.matmul(
                        pv[:], wv_sbs[g][:, cbl * 128 : (cbl + 1) * 128], xts[cb][:],
                        start=st, stop=sp, skip_group_check=True,
                    )
                    yield
                rope(pk, KT[bi], i4)
                vt = wkp.tile([128, 512], f32, tag="vt", bufs=2, name="vt")
                nc.vector.tensor_copy(vt[:], pv[:])
                yield
                for jj in range(4):
                    ptp = pp.tile([128, 512], f32, tag="s", bufs=2, name="ptp")
                    nc.tensor.transpose(ptp[:, 0:128], vt[:, jj * 128 : (jj + 1) * 128], id_sb[:])
                    jb = i4 * 4 + jj
                    nc.vector.tensor_copy(Vn[bi][:, jb * 128 : (jb + 1) * 128], ptp[:, 0:128])
                    yield

            def strip(bi, i4, h):
                """Causal attention rows [i4*512, (i4+1)*512) for head h,
                with a two-step lag so the PE never waits on the exp."""
                qs = slice(i4 * 512, (i4 + 1) * 512)
                njs = 4 * (i4 + 1)
                pav = pp.tile([128, 512], f32, tag="av", bufs=2, name="pav")
                pden = pp.tile([1, 512], f32, tag="den", bufs=1, name="pden")
                pend = []
                for j in range(njs):
                    diag = j - 4 * i4
                    off = max(diag, 0) * 128
                    pst = pp.tile([128, 512], f32, tag="s", bufs=2, name="pst")
                    nc.tensor.matmul(
                        pst[:, off:512],
                        KT[bi][:, j * 128 : (j + 1) * 128],
                        QT[bi][h][:, i4 * 512 + off : (i4 + 1) * 512],
                        start=True, stop=True, skip_group_check=True,
                    )
                    E = wkp.tile([128, 512], bf, tag="E", bufs=4, name="E")
                    nc.scalar.activation(E[:, off:512], pst[:, off:512], EXP, scale=F32R_SCALE)
                    if diag >= 0:
                        nc.vector.tensor_mul(
                            E[:, off : off + 128], E[:, off : off + 128], mask_sb[:]
                        )
                    pend.append((j, E, off))
                    if len(pend) > 1:
                        emit_pden_pav(bi, pden, pav, pend.pop(0), njs)
                    yield
                while pend:
                    emit_pden_pav(bi, pden, pav, pend.pop(0), njs)
                    yield
                rec = wkp.tile([1, 512], f32, tag="rec", bufs=2, name="rec")
                nc.vector.reciprocal_approx_fast(rec[:], pden[:])
                rbc = wkp.tile([128, 512], f32, tag="rbc", bufs=2, name="rbc")
                nc.gpsimd.partition_broadcast(rbc[:], rec[:])
                nc.vector.tensor_mul(AT[bi][h][:, qs], pav[:], rbc[:])
                yield

            def emit_pden_pav(bi, pden, pav, item, njs):
                pj, pE, poff = item
                nc.tensor.matmul(
                    pden[:, poff:512], ones_sb[:, 0:1], pE[:, poff:512],
                    start=(pj == 0), stop=(pj == njs - 1), skip_group_check=True,
                )
                nc.tensor.matmul(
                    pav[:, poff:512],
                    Vn[bi][:, pj * 128 : (pj + 1) * 128], pE[:, poff:512],
                    start=(pj == 0), stop=(pj == njs - 1), skip_group_check=True,
                )

            def strip_pair(bi, i4):
                for h in range(HL):
                    for _ in strip(bi, i4, h):
                        yield

            def po_gen(bi, i4, alt=False):
                """Output projection for the 4 row-blocks of quarter i4.
                alt=True double-banks via the idle "s" ring (bare-tail mode)."""
                gi = 0
                for it in range(i4 * 4, (i4 + 1) * 4):
                    po_sb = wkp.tile([128, c], bf, tag="yout", bufs=2, name="po_sb")
                    for n in range(c // 512):
                        if alt and gi % 2:
                            po = pp.tile([128, 512], f32, tag="s", bufs=2, name="po")
                        else:
                            po = pp.tile([128, 512], f32, tag="po", bufs=1, name="po")
                        gi += 1
                        for hh in range(HL):
                            nc.tensor.matmul(
                                po[:],
                                AT[bi][hh][:, it * 128 : (it + 1) * 128],
                                wp_sb[:, hh * c + n * 512 : hh * c + (n + 1) * 512],
                                start=(hh == 0), stop=(hh == HL - 1),
                                skip_group_check=True,
                            )
                        if n % 2:
                            nc.scalar.copy(po_sb[:, n * 512 : (n + 1) * 512], po[:])
                        else:
                            nc.vector.tensor_copy(po_sb[:, n * 512 : (n + 1) * 512], po[:])
                        yield
                    nc.sync.dma_start(y[bi, it * 128 : (it + 1) * 128, :], po_sb[:])
                    yield

            def weave(gens):
                """Advance generators proportionally to their PE-time strides."""
                hq = [(0.0, i) for i in range(len(gens))]
                heapq.heapify(hq)
                while hq:
                    vt_, i = heapq.heappop(hq)
                    g, stride = gens[i]
                    try:
                        next(g)
                        heapq.heappush(hq, (vt_ + stride, i))
                    except StopIteration:
                        pass

            # ---- unified software pipeline over (batch, quarter) ----
            # block schedule: proj(b,k) ∥ strip_pair lagging 1 block ∥
            # out-proj lagging 2 blocks; strips ascend so attention for
            # quarter i4 starts as soon as K/V blocks 0..4(i4+1) exist.
            stages = [(0, k) for k in range(nt)] + [(1, k) for k in range(nt)] + [None, None]
            for si, pj in enumerate(stages):
                gens = []
                if pj is not None:
                    gens.append((proj_quarter(*pj, dual_ring=(si == 0)), 0.6))
                if si >= 1 and stages[si - 1] is not None:
                    gens.append((strip_pair(*stages[si - 1]), 0.9))
                if si >= 2 and stages[si - 2] is not None:
                    bi2, k2 = stages[si - 2]
                    gens.append((po_gen(bi2, k2, alt=(si == len(stages) - 1)), 0.7))
                weave(gens)

            if debug:
                for bi in range(b):
                    for h in range(HL):
                        nc.sync.dma_start(dbg_qt[bi, h], QT[bi][h][:])
                        nc.sync.dma_start(dbg_at[bi, h], AT[bi][h][:])
                    nc.sync.dma_start(dbg_kt[bi], KT[bi][:])
                    nc.sync.dma_start(dbg_vn[bi], Vn[bi][:])

    nc.compile()
    return nc


def host_inputs(x, Wq, Wk, Wv, Wp, ncores=NCORES, mmdt="bf16"):
    import ml_dtypes

    mdt = np.float32 if mmdt == "f32r" else ml_dtypes.bfloat16
    b, t, c = x.shape
    d = D
    xT = np.ascontiguousarray(np.transpose(x, (0, 2, 1)))  # [B, C, T]
    inv = (1.0 / (10000.0 ** (np.arange(0, d, 2, dtype=np.float32) / np.float32(d)))).astype(np.float32)
    pos = np.arange(t, dtype=np.float32)
    fr = np.outer(pos, inv).astype(np.float32)  # [T, 64]
    cosT = np.cos(fr).T.astype(np.float32)  # [64, T]
    sinT = np.sin(fr).T.astype(np.float32)
    # pair-interleaved rope tables: partition 2m,2m+1 <- freq m; sign -/+ on sin
    cosI = np.ascontiguousarray(np.repeat(cosT, 2, axis=0))  # [128, T]
    sinS = np.ascontiguousarray(np.stack([-sinT, sinT], axis=1).reshape(128, t))
    # column permutation putting rope pair (m, m+64) at (2m, 2m+1), per head
    perm = np.stack([np.arange(64), np.arange(64) + 64], 1).reshape(128)
    maskf = np.ascontiguousarray(np.triu(np.ones((128, 128), np.float32)))
    onesv = np.ones((128, 1), np.float32)
    ident = np.eye(128, dtype=np.float32)

    def permute_heads(w):
        nh = w.shape[1] // d
        wv_ = w.reshape(w.shape[0], nh, d)
        return np.ascontiguousarray(wv_[:, :, perm].reshape(w.shape))

    Wq_p = permute_heads(Wq)
    Wk_p = permute_heads(Wk)

    def chunk128(w):
        # [c, m] -> [128, (c//128) * m]: SBUF layout, linear DMA
        cc, mm = w.shape
        return np.ascontiguousarray(
            w.reshape(cc // 128, 128, mm).transpose(1, 0, 2).reshape(128, -1)
        )

    xTm = xT.astype(mdt) if mdt is not np.float32 else xT
    in_maps = []
    for ci in range(ncores):
        qs = slice(ci * HL * d, (ci + 1) * HL * d)
        in_maps.append(
            {
                "xT": xTm,
                "wq": chunk128(Wq_p[:, qs]).astype(mdt),
                "wk": chunk128(Wk_p[:, ci * d : (ci + 1) * d]).astype(mdt),
                "wv": chunk128(Wv[:, ci * d : (ci + 1) * d]).astype(mdt),
                "wp": chunk128(Wp[qs, :]).astype(mdt),
                "cos2": cosI,
                "sin2": sinS,
                "maskf": maskf.astype(mdt),
                "onesv": onesv.astype(mdt),
                "ident": ident,
            }
        )
    return in_maps


_NC_CACHE = {}

MMDT = "bf16"


def _get_nc(mmdt=None):
    mmdt = mmdt or MMDT
    key = (B, T, C, mmdt)
    if key not in _NC_CACHE:
        _NC_CACHE[key] = build_nc(B, T, C, mmdt=mmdt)
    return _NC_CACHE[key]


def _install_cc_error_surfacing():
    """Make neuronx_cc hook failures print a real traceback instead of the
    opaque PJRT 'py_result' error."""
    try:
        from concourse import bass2jax

        bass2jax.install_neuronx_cc_hook()
        import libneuronxla

        if getattr(libneuronxla, "_tb_wrapped", False):
            return
        inner = libneuronxla.neuronx_cc

        def wrapped(*a, **k):
            try:
                return inner(*a, **k)
            except BaseException:
                import traceback

                traceback.print_exc()
                raise

        libneuronxla.neuronx_cc = wrapped
        libneuronxla._tb_wrapped = True
    except Exception:
        pass


def run_spmd(x, Wq, Wk, Wv, Wp, trace=False, mmdt=None):
    from concourse.bass_utils import run_bass_kernel_spmd

    mmdt = mmdt or MMDT
    _install_cc_error_surfacing()

    nc = _get_nc(mmdt)
    in_maps = host_inputs(x, Wq, Wk, Wv, Wp, mmdt=mmdt)
    last_err = None
    for attempt in range(3):
        try:
            res = run_bass_kernel_spmd(
                nc, in_maps, core_ids=list(range(NCORES)), trace=trace
            )
            break
        except Exception as e:  # transient NRT device faults: retry
            last_err = e
            import time as _time

            _time.sleep(5.0)
    else:
        raise last_err
    acc = res.results[0]["y"].astype(np.float64)
    for i in range(1, NCORES):
        acc += res.results[i]["y"].astype(np.float64)
    return acc.astype(np.float32), res


def kernel(x, Wq, Wk, Wv, Wp):
    out, _ = run_spmd(x, Wq, Wk, Wv, Wp, trace=False)
    return out


# revision 31
# speedup vs baseline: 1.1830x; 1.1830x over previous
import sys

if "/opt/trn_rl_repo" not in sys.path:
    sys.path.insert(0, "/opt/trn_rl_repo")

import heapq

import numpy as np

B, T, C = 2, 2048, 2048
H, H_KV = 16, 8
D = C // H  # 128
NCORES = 8
HL = H // NCORES  # 2 local query heads per core; 1 kv head per core

F32R_SCALE = 0.08838834764831845  # 1/sqrt(128)


def build_nc(b=B, t=T, c=C, mmdt="bf16", debug=False):
    """Per-core Bass program. Same program on all 8 cores; the sharding
    lives entirely in the input data each core receives.

    Schedule (all engines in-order, so emission order = execution order):
      phase A: proj(b0) quarter by quarter (qq pass then kv pass per i4)
      phase B: attention strips of b0 (descending i4) woven with proj(b1)
               quarters and b0 output-projection blocks
      phase C: attention strips of b1 (descending i4) woven with the
               remaining output-projection blocks
    The weave keeps the PE queue stocked with independent matmuls so
    dependency stalls (exp latency, psum evictions) never idle the PE.
    """
    import concourse.bass as bass  # noqa: F401
    import concourse.mybir as mybir
    import concourse.tile as tile
    from concourse import bacc

    f32 = mybir.dt.float32
    bf = mybir.dt.float32r if mmdt == "f32r" else mybir.dt.bfloat16
    EXP = mybir.ActivationFunctionType.Exp

    ncb = c // 128  # contraction blocks for projections
    nt = t // 512  # number of 512-wide t quarters
    swap_mask = [i ^ 1 for i in range(32)]

    nc = bacc.Bacc("TRN2", target_bir_lowering=False, debug=False)

    # weights arrive pre-laid-out for SBUF: [128 partitions, chunked cols]
    xT = nc.dram_tensor("xT", [b, c, t], bf, kind="ExternalInput")
    wq = nc.dram_tensor("wq", [128, (c // 128) * HL * D], bf, kind="ExternalInput")
    wk = nc.dram_tensor("wk", [128, (c // 128) * D], bf, kind="ExternalInput")
    wv = nc.dram_tensor("wv", [128, (c // 128) * D], bf, kind="ExternalInput")
    wp = nc.dram_tensor("wp", [128, HL * c], bf, kind="ExternalInput")
    cos2 = nc.dram_tensor("cos2", [128, t], f32, kind="ExternalInput")
    sin2 = nc.dram_tensor("sin2", [128, t], f32, kind="ExternalInput")
    maskf = nc.dram_tensor("maskf", [128, 128], bf, kind="ExternalInput")
    onesv = nc.dram_tensor("onesv", [128, 1], bf, kind="ExternalInput")
    ident = nc.dram_tensor("ident", [128, 128], f32, kind="ExternalInput")
    y = nc.dram_tensor("y", [b, t, c], bf, kind="ExternalOutput")
    if debug:
        dbg_qt = nc.dram_tensor("dbg_qt", [b, HL, 128, t], bf, kind="ExternalOutput")
        dbg_kt = nc.dram_tensor("dbg_kt", [b, 128, t], bf, kind="ExternalOutput")
        dbg_vn = nc.dram_tensor("dbg_vn", [b, 128, t], bf, kind="ExternalOutput")
        dbg_at = nc.dram_tensor("dbg_at", [b, HL, 128, t], bf, kind="ExternalOutput")

    with tile.TileContext(nc) as tc:
        with (
            tc.tile_pool(name="wts", bufs=1) as wpool,
            tc.tile_pool(name="data", bufs=1) as dpool,
            tc.tile_pool(name="work", bufs=2) as wkp,
            tc.tile_pool(name="psum", bufs=1, space="PSUM") as pp,
        ):
            # ---- weight/table DMAs, in first-use priority order ----
            nw = 4  # cb chunks per weight DMA group
            ngrp = ncb // nw
            wq_sbs = [wpool.tile([128, nw * HL * D], bf, name=f"wq{g}") for g in range(ngrp)]
            wk_sbs = [wpool.tile([128, nw * D], bf, name=f"wk{g}") for g in range(ngrp)]
            wv_sbs = [wpool.tile([128, nw * D], bf, name=f"wv{g}") for g in range(ngrp)]
            cosq = [wpool.tile([128, 512], f32, name=f"cosq{i}") for i in range(nt)]
            sinq = [wpool.tile([128, 512], f32, name=f"sinq{i}") for i in range(nt)]
            id_sb = wpool.tile([128, 128], f32)
            mask_sb = wpool.tile([128, 128], bf)
            ones_sb = wpool.tile([128, 1], bf)
            wp_sb = wpool.tile([128, HL * c], bf)

            def dma_wgrp(g):
                nc.scalar.dma_start(
                    wq_sbs[g][:], wq[:, g * nw * HL * D : (g + 1) * nw * HL * D]
                )
                nc.scalar.dma_start(wk_sbs[g][:], wk[:, g * nw * D : (g + 1) * nw * D])
                nc.scalar.dma_start(wv_sbs[g][:], wv[:, g * nw * D : (g + 1) * nw * D])

            # first 256 cols of wq unblock the PE earliest; wk/wv follow
            # since the kv pass trails the qq pass by a full quarter
            nc.scalar.dma_start(wq_sbs[0][:, 0:256], wq[:, 0:256])
            nc.scalar.dma_start(wq_sbs[0][:, 256:], wq[:, 256 : nw * HL * D])
            nc.scalar.dma_start(wq_sbs[1][:], wq[:, nw * HL * D : 2 * nw * HL * D])
            nc.scalar.dma_start(wk_sbs[0][:], wk[:, 0 : nw * D])
            nc.scalar.dma_start(wv_sbs[0][:], wv[:, 0 : nw * D])
            for g in (2, 3):
                nc.scalar.dma_start(
                    wq_sbs[g][:], wq[:, g * nw * HL * D : (g + 1) * nw * HL * D]
                )
            nc.scalar.dma_start(cosq[0][:], cos2[:, 0:512])
            nc.scalar.dma_start(sinq[0][:], sin2[:, 0:512])
            for g in range(1, ngrp):
                nc.scalar.dma_start(wk_sbs[g][:], wk[:, g * nw * D : (g + 1) * nw * D])
                nc.scalar.dma_start(wv_sbs[g][:], wv[:, g * nw * D : (g + 1) * nw * D])
            nc.scalar.dma_start(id_sb[:], ident[:, :])
            for i in range(1, nt):
                nc.scalar.dma_start(cosq[i][:], cos2[:, i * 512 : (i + 1) * 512])
                nc.scalar.dma_start(sinq[i][:], sin2[:, i * 512 : (i + 1) * 512])
            nc.scalar.dma_start(mask_sb[:], maskf[:, :])
            nc.scalar.dma_start(ones_sb[:], onesv[:, :])
            nc.scalar.dma_start(wp_sb[:], wp[:, :])
            warm = wpool.tile([128, 1], f32)
            nc.scalar.activation(warm[:], id_sb[:, 0:1], EXP, scale=1.0)

            # ---- per-batch persistent tiles ----
            QT = [[dpool.tile([128, t], bf, name=f"QT{bi}_{h}") for h in range(HL)] for bi in range(b)]
            KT = [dpool.tile([128, t], bf, name=f"KT{bi}") for bi in range(b)]
            Vn = [dpool.tile([128, t], bf, name=f"Vn{bi}") for bi in range(b)]
            AT = [[dpool.tile([128, t], bf, name=f"AT{bi}_{h}") for h in range(HL)] for bi in range(b)]

            def rope(ps, dest, i4):
                """dest[:, quarter] = ps*cos + swap_adjacent(ps)*sin"""
                ts_ = slice(i4 * 512, (i4 + 1) * 512)
                ra = wkp.tile([128, 512], f32, tag="ra", bufs=2, name="ra")
                rb = wkp.tile([128, 512], f32, tag="rb", bufs=2, name="rb")
                nc.vector.tensor_mul(ra[:], ps[:], cosq[i4][:])
                nc.vector.stream_shuffle(rb[:], ps[:], swap_mask)
                nc.vector.tensor_mul(rb[:], rb[:], sinq[i4][:])
                nc.vector.tensor_add(dest[:, ts_], ra[:], rb[:])

            def proj_quarter(bi, i4, dual_ring=False):
                """Project x[:, quarter i4] -> QT (roped), KT (roped), Vn."""
                ts_ = slice(i4 * 512, (i4 + 1) * 512)
                xts = []
                for cb in range(ncb):
                    xtp = wkp.tile([128, 512], bf, tag="xt", bufs=32, name=f"xt{bi}_{i4}_{cb}")
                    eng = nc.gpsimd if dual_ring and cb % 2 else nc.sync
                    eng.dma_start(xtp[:], xT[bi, cb * 128 : (cb + 1) * 128, ts_])
                    xts.append(xtp)
                # pass 1: the two q heads
                pq0 = pp.tile([128, 512], f32, tag="proj", bufs=2, name="pq0")
                pq1 = pp.tile([128, 512], f32, tag="proj", bufs=2, name="pq1")
                for cb in range(ncb):
                    g, cbl = cb // nw, cb % nw
                    base = cbl * HL * D
                    st, sp = (cb == 0), (cb == ncb - 1)
                    nc.tensor.matmul(
                        pq0[:], wq_sbs[g][:, base : base + 128], xts[cb][:],
                        start=st, stop=sp, skip_group_check=True,
                    )
                    nc.tensor.matmul(
                        pq1[:], wq_sbs[g][:, base + 128 : base + 256], xts[cb][:],
                        start=st, stop=sp, skip_group_check=True,
                    )
                    yield
                rope(pq0, QT[bi][0], i4)
                rope(pq1, QT[bi][1], i4)
                yield
                # pass 2: k (roped) and v (transposed into Vn)
                pk = pp.tile([128, 512], f32, tag="proj", bufs=2, name="pk")
                pv = pp.tile([128, 512], f32, tag="proj", bufs=2, name="pv")
                for cb in range(ncb):
                    g, cbl = cb // nw, cb % nw
                    st, sp = (cb == 0), (cb == ncb - 1)
                    nc.tensor.matmul(
                        pk[:], wk_sbs[g][:, cbl * 128 : (cbl + 1) * 128], xts[cb][:],
                        start=st, stop=sp, skip_group_check=True,
                    )
                    nc.tensor.matmul(
                        pv[:], wv_sbs[g][:, cbl * 128 : (cbl + 1) * 128], xts[cb][:],
                        start=st, stop=sp, skip_group_check=True,
                    )
                    yield
                rope(pk, KT[bi], i4)
                vt = wkp.tile([128, 512], f32, tag="vt", bufs=2, name="vt")
                nc.vector.tensor_copy(vt[:], pv[:])
                yield
                for jj in range(4):
                    ptp = pp.tile([128, 512], f32, tag="s", bufs=2, name="ptp")
                    nc.tensor.transpose(ptp[:, 0:128], vt[:, jj * 128 : (jj + 1) * 128], id_sb[:])
                    jb = i4 * 4 + jj
                    nc.vector.tensor_copy(Vn[bi][:, jb * 128 : (jb + 1) * 128], ptp[:, 0:128])
                    yield

            def strip(bi, i4, h):
                """Causal attention rows [i4*512, (i4+1)*512) for head h,
                with a two-step lag so the PE never waits on the exp."""
                qs = slice(i4 * 512, (i4 + 1) * 512)
                njs = 4 * (i4 + 1)
                pav = pp.tile([128, 512], f32, tag="av", bufs=2, name="pav")
                pden = pp.tile([1, 512], f32, tag="den", bufs=1, name="pden")
                pend = []
                for j in range(njs):
                    diag = j - 4 * i4
                    off = max(diag, 0) * 128
                    pst = pp.tile([128, 512], f32, tag="s", bufs=2, name="pst")
                    nc.tensor.matmul(
                        pst[:, off:512],
                        KT[bi][:, j * 128 : (j + 1) * 128],
                        QT[bi][h][:, i4 * 512 + off : (i4 + 1) * 512],
                        start=True, stop=True, skip_group_check=True,
                    )
                    E = wkp.tile([128, 512], bf, tag="E", bufs=4, name="E")
                    nc.scalar.activation(E[:, off:512], pst[:, off:512], EXP, scale=F32R_SCALE)
                    if diag >= 0:
                        nc.vector.tensor_mul(
                            E[:, off : off + 128], E[:, off : off + 128], mask_sb[:]
                        )
                    pend.append((j, E, off))
                    if len(pend) > 1:
                        emit_pden_pav(bi, pden, pav, pend.pop(0), njs)
                    yield
                while pend:
                    emit_pden_pav(bi, pden, pav, pend.pop(0), njs)
                    yield
                rec = wkp.tile([1, 512], f32, tag="rec", bufs=2, name="rec")
                nc.vector.reciprocal_approx_fast(rec[:], pden[:])
                rbc = wkp.tile([128, 512], f32, tag="rbc", bufs=2, name="rbc")
                nc.gpsimd.partition_broadcast(rbc[:], rec[:])
                nc.vector.tensor_mul(AT[bi][h][:, qs], pav[:], rbc[:])
                yield

            def emit_pden_pav(bi, pden, pav, item, njs):
                pj, pE, poff = item
                nc.tensor.matmul(
                    pden[:, poff:512], ones_sb[:, 0:1], pE[:, poff:512],
                    start=(pj == 0), stop=(pj == njs - 1), skip_group_check=True,
                )
                nc.tensor.matmul(
                    pav[:, poff:512],
                    Vn[bi][:, pj * 128 : (pj + 1) * 128], pE[:, poff:512],
                    start=(pj == 0), stop=(pj == njs - 1), skip_group_check=True,
                )

            def strip_pair(bi, i4):
                for h in range(HL):
                    for _ in strip(bi, i4, h):
                        yield

            def po_gen(bi, i4, alt=False):
                """Output projection for the 4 row-blocks of quarter i4.
                alt=True double-banks via the idle "s" ring (bare-tail mode)."""
                gi = 0
                for it in range(i4 * 4, (i4 + 1) * 4):
                    po_sb = wkp.tile([128, c], bf, tag="yout", bufs=2, name="po_sb")
                    for n in range(c // 512):
                        if alt and gi % 2:
                            po = pp.tile([128, 512], f32, tag="s", bufs=2, name="po")
                        else:
                            po = pp.tile([128, 512], f32, tag="po", bufs=1, name="po")
                        gi += 1
                        for hh in range(HL):
                            nc.tensor.matmul(
                                po[:],
                                AT[bi][hh][:, it * 128 : (it + 1) * 128],
                                wp_sb[:, hh * c + n * 512 : hh * c + (n + 1) * 512],
                                start=(hh == 0), stop=(hh == HL - 1),
                                skip_group_check=True,
                            )
                        if n % 2:
                            nc.scalar.copy(po_sb[:, n * 512 : (n + 1) * 512], po[:])
                        else:
                            nc.vector.tensor_copy(po_sb[:, n * 512 : (n + 1) * 512], po[:])
                        yield
                    nc.sync.dma_start(y[bi, it * 128 : (it + 1) * 128, :], po_sb[:])
                    yield

            def weave(gens):
                """Advance generators proportionally to their PE-time strides."""
                hq = [(0.0, i) for i in range(len(gens))]
                heapq.heapify(hq)
                while hq:
                    vt_, i = heapq.heappop(hq)
                    g, stride = gens[i]
                    try:
                        next(g)
                        heapq.heappush(hq, (vt_ + stride, i))
                    except StopIteration:
                        pass

            # ---- unified software pipeline over (batch, quarter) ----
            # block schedule: proj(b,k) ∥ strip_pair lagging 1 block ∥
            # out-proj lagging 2 blocks; strips ascend so attention for
            # quarter i4 starts as soon as K/V blocks 0..4(i4+1) exist.
            # Strides are per-block: each generator spans the whole block
            # so no stream drains early and leaves the others bare.
            stages = [(0, k) for k in range(nt)] + [(1, k) for k in range(nt)] + [None, None]
            for si, pj in enumerate(stages):
                specs = []  # (gen, n_steps, pe_us)
                if pj is not None:
                    specs.append((proj_quarter(*pj, dual_ring=(si == 0)), 38, 19.0))
                if si >= 1 and stages[si - 1] is not None:
                    bi1, k1 = stages[si - 1]
                    njs = 4 * (k1 + 1)
                    specs.append((strip_pair(bi1, k1), 2 * njs + 4, 0.58 * (3 * njs + 2)))
                if si >= 2 and stages[si - 2] is not None:
                    bi2, k2 = stages[si - 2]
                    specs.append((po_gen(bi2, k2, alt=(si == len(stages) - 1)), 20, 11.0))
                tb = max(pe for _, _, pe in specs)
                weave([(g, tb / steps) for g, steps, pe in specs])

            if debug:
                for bi in range(b):
                    for h in range(HL):
                        nc.sync.dma_start(dbg_qt[bi, h], QT[bi][h][:])
                        nc.sync.dma_start(dbg_at[bi, h], AT[bi][h][:])
                    nc.sync.dma_start(dbg_kt[bi], KT[bi][:])
                    nc.sync.dma_start(dbg_vn[bi], Vn[bi][:])

    nc.compile()
    return nc


def host_inputs(x, Wq, Wk, Wv, Wp, ncores=NCORES, mmdt="bf16"):
    import ml_dtypes

    mdt = np.float32 if mmdt == "f32r" else ml_dtypes.bfloat16
    b, t, c = x.shape
    d = D
    xT = np.ascontiguousarray(np.transpose(x, (0, 2, 1)))  # [B, C, T]
    inv = (1.0 / (10000.0 ** (np.arange(0, d, 2, dtype=np.float32) / np.float32(d)))).astype(np.float32)
    pos = np.arange(t, dtype=np.float32)
    fr = np.outer(pos, inv).astype(np.float32)  # [T, 64]
    cosT = np.cos(fr).T.astype(np.float32)  # [64, T]
    sinT = np.sin(fr).T.astype(np.float32)
    # pair-interleaved rope tables: partition 2m,2m+1 <- freq m; sign -/+ on sin
    cosI = np.ascontiguousarray(np.repeat(cosT, 2, axis=0))  # [128, T]
    sinS = np.ascontiguousarray(np.stack([-sinT, sinT], axis=1).reshape(128, t))
    # column permutation putting rope pair (m, m+64) at (2m, 2m+1), per head
    perm = np.stack([np.arange(64), np.arange(64) + 64], 1).reshape(128)
    maskf = np.ascontiguousarray(np.triu(np.ones((128, 128), np.float32)))
    onesv = np.ones((128, 1), np.float32)
    ident = np.eye(128, dtype=np.float32)

    def permute_heads(w):
        nh = w.shape[1] // d
        wv_ = w.reshape(w.shape[0], nh, d)
        return np.ascontiguousarray(wv_[:, :, perm].reshape(w.shape))

    Wq_p = permute_heads(Wq)
    Wk_p = permute_heads(Wk)

    def chunk128(w):
        # [c, m] -> [128, (c//128) * m]: SBUF layout, linear DMA
        cc, mm = w.shape
        return np.ascontiguousarray(
            w.reshape(cc // 128, 128, mm).transpose(1, 0, 2).reshape(128, -1)
        )

    xTm = xT.astype(mdt) if mdt is not np.float32 else xT
    in_maps = []
    for ci in range(ncores):
        qs = slice(ci * HL * d, (ci + 1) * HL * d)
        in_maps.append(
            {
                "xT": xTm,
                "wq": chunk128(Wq_p[:, qs]).astype(mdt),
                "wk": chunk128(Wk_p[:, ci * d : (ci + 1) * d]).astype(mdt),
                "wv": chunk128(Wv[:, ci * d : (ci + 1) * d]).astype(mdt),
                "wp": chunk128(Wp[qs, :]).astype(mdt),
                "cos2": cosI,
                "sin2": sinS,
                "maskf": maskf.astype(mdt),
                "onesv": onesv.astype(mdt),
                "ident": ident,
            }
        )
    return in_maps


_NC_CACHE = {}

MMDT = "bf16"


def _get_nc(mmdt=None):
    mmdt = mmdt or MMDT
    key = (B, T, C, mmdt)
    if key not in _NC_CACHE:
        _NC_CACHE[key] = build_nc(B, T, C, mmdt=mmdt)
    return _NC_CACHE[key]


def _install_cc_error_surfacing():
    """Make neuronx_cc hook failures print a real traceback instead of the
    opaque PJRT 'py_result' error."""
    try:
        from concourse import bass2jax

        bass2jax.install_neuronx_cc_hook()
        import libneuronxla

        if getattr(libneuronxla, "_tb_wrapped", False):
            return
        inner = libneuronxla.neuronx_cc

        def wrapped(*a, **k):
            try:
                return inner(*a, **k)
            except BaseException:
                import traceback

                traceback.print_exc()
                raise

        libneuronxla.neuronx_cc = wrapped
        libneuronxla._tb_wrapped = True
    except Exception:
        pass


def run_spmd(x, Wq, Wk, Wv, Wp, trace=False, mmdt=None):
    from concourse.bass_utils import run_bass_kernel_spmd

    mmdt = mmdt or MMDT
    _install_cc_error_surfacing()

    nc = _get_nc(mmdt)
    in_maps = host_inputs(x, Wq, Wk, Wv, Wp, mmdt=mmdt)
    last_err = None
    for attempt in range(3):
        try:
            res = run_bass_kernel_spmd(
                nc, in_maps, core_ids=list(range(NCORES)), trace=trace
            )
            break
        except Exception as e:  # transient NRT device faults: retry
            last_err = e
            import time as _time

            _time.sleep(5.0)
    else:
        raise last_err
    acc = res.results[0]["y"].astype(np.float64)
    for i in range(1, NCORES):
        acc += res.results[i]["y"].astype(np.float64)
    return acc.astype(np.float32), res


def kernel(x, Wq, Wk, Wv, Wp):
    out, _ = run_spmd(x, Wq, Wk, Wv, Wp, trace=False)
    return out
